# revision 11
# baseline (speedup 1.0000x reference)
"""Trainium2 Bass kernel for CrossAttentionModule (channel-wise attention).

Math restructuring
------------------
Reference (per sample b, with n = H*W pixels, C channels):
    q = Wq @ fm + bq            # [C, n]
    k = Wk * am + bk            # [C, n]  (rank-2 in the channel axis!)
    v = Wv @ fm + bv            # [C, n]
    scores[i, j] = <q[i, :], k[j, :]>
    out = softmax_j(scores) @ v
    result = gamma * out + fm

Because k[j, p] = Wk[j] * am[p] + bk[j]:
    scores[i, j] = s1[i] * Wk[j] + s2[i] * bk[j]
where
    s1 = Wq @ (fm @ am) + sum(am) * bq      # [C]
    s2 = Wq @ (fm @ 1)  + n * bq            # [C]

Device pipeline (per core = one sample, data-parallel over batch):
  phase A (8 rounds x 2 o-chunks, c-outer):
    round 0 streams fm (kept resident in SBUF, fp32); DVE computes the
    u = [fm@am, fm@1] reductions per c-chunk.  Every round runs the
    V = Wv@fm GEMM for its 2 o-chunks (f32r), accumulating over all 16
    c-chunks; v tiles written bf16 (+ones cols for Z).  The s matvec
    streams Wq^T as the f32r MOVING operand against the tiny stationary
    u (2 cols -> trivial weight loads), one 512-wide o-quarter per 2
    rounds, so the 16MB Wq read hides under the V GEMM.
  m phase: row max via direction-sampled support points of
    {(Wk_j, bk_j)} evaluated with tiny rank-2 PE matmuls; B = [s1; s2; m]
    rows assembled in SBUF (m via a small DRAM transpose roundtrip).
  phase D (4 i-blocks): scores^T tiles come from rank-3 PE matmuls
    A^T @ B with A = [wk; bk; -1] (no DVE work), ACT exp -> bf16 e
    tiles, then probs @ v accumulates over j on PE; epilogue divides by
    the ones-column Z, scales by gamma and adds the resident fm.
"""

import os
import sys

for _p in ("/opt/trn_rl_repo", "/root/.axon_site/_ro/trn_rl_repo"):
    if os.path.isdir(_p) and _p not in sys.path:
        sys.path.insert(0, _p)

from contextlib import ExitStack

import numpy as np

import concourse.bacc as bacc
import concourse.bass as bass
import concourse.mybir as mybir
import concourse.tile as tile

C = 2048
NPIX = 1024
NCORES = 8
NH = 64  # direction-sampled support points for the row max
NCHUNK = C // 128  # 16

F32 = mybir.dt.float32
F32R = mybir.dt.float32r
BF16 = mybir.dt.bfloat16
OP = mybir.AluOpType
AX = mybir.AxisListType
AF = mybir.ActivationFunctionType

# dtype of the probs/v operands for the P@V GEMM (bf16 halves SBUF and
# enables fast weight load; rel-err contribution ~6e-4).
MM_DT = BF16 if os.environ.get("CA_MM_DT", "bf16") == "bf16" else F32R

# n-chunk split of the 1026-wide (v | ones | pad) moving operand: each
# matmul output must fit one PSUM bank (<=512 fp32).  Column 1024 holds
# the ones-column (Z); 1025 is padding.
NSPLIT = [(0, 342), (342, 684), (684, 1026)]

# phase A rounds: (o_start, count) pairs; 2 o-chunks per round so V GEMM
# needs only 4 PSUM banks and can start after the first fm tile lands.
ROUNDS = [(2 * r, 2) for r in range(8)]
# s-matvec schedule: (round, quarter, c_range, start, stop)
SSCHED = {
    1: (0, range(0, 8), True, False),
    2: (0, range(8, 16), False, True),
    3: (1, range(0, 8), True, False),
    4: (1, range(8, 16), False, True),
    5: (2, range(0, 8), True, False),
    6: (2, range(8, 16), False, True),
    7: (3, range(0, 16), True, True),
}


def build_nc(mm_dt=MM_DT, passes=1):
    nc = bacc.Bacc("TRN2", target_bir_lowering=False)

    fm = nc.declare_dram_parameter("fm", [C, NPIX], F32, isOutput=False)
    am = nc.declare_dram_parameter("am", [1, NPIX], F32, isOutput=False)
    # weight blocks pre-swizzled on host: [o, p, c, f] = Wv.T[c*128+p, o*128+f]
    wvt = nc.declare_dram_parameter("wvt", [NCHUNK, 128, NCHUNK, 128], F32, isOutput=False)
    # wqm[cb, p, o] = Wq[o, cb*128+p]  (moving-operand layout, c on partitions)
    wqm = nc.declare_dram_parameter("wqm", [NCHUNK, 128, C], F32, isOutput=False)
    arows = nc.declare_dram_parameter("arows", [3, C], F32, isOutput=False)  # wk, bk, -1
    brows = nc.declare_dram_parameter("brows", [2, C], F32, isOutput=False)  # bq, n*bq
    bvcol = nc.declare_dram_parameter("bvcol", [128, NCHUNK], F32, isOutput=False)
    hull = nc.declare_dram_parameter("hull", [2, NH], F32, isOutput=False)
    gam = nc.declare_dram_parameter("gamma", [1, 1], F32, isOutput=False)
    out = nc.declare_dram_parameter("out", [C, NPIX], F32, isOutput=True)

    with ExitStack() as ctx:
        tc = ctx.enter_context(tile.TileContext(nc))
        small = ctx.enter_context(tc.tile_pool(name="small", bufs=1))
        dramp = ctx.enter_context(tc.tile_pool(name="dram", bufs=1, space="DRAM"))

        # ---- small persistent tiles -------------------------------------
        arows_t = small.tile([3, C], F32R, tag="arows")
        nc.gpsimd.dma_start(out=arows_t[:], in_=arows[:].bitcast(F32R))
        brows_t = small.tile([2, C], F32, tag="brows")
        nc.gpsimd.dma_start(out=brows_t[:], in_=brows[:])
        hull_t = small.tile([2, NH], F32R, tag="hull")
        nc.gpsimd.dma_start(out=hull_t[:], in_=hull[:].bitcast(F32R))
        bv_t = small.tile([128, NCHUNK], F32, tag="bv")
        nc.gpsimd.dma_start(out=bv_t[:], in_=bvcol[:])
        gam_bc = small.tile([128, 1], F32, tag="gam")
        nc.gpsimd.dma_start(out=gam_bc[:], in_=gam[:].to_broadcast([128, 1]))
        am_bc = small.tile([128, NPIX], F32, tag="am_bc")
        nc.gpsimd.dma_start(out=am_bc[:], in_=am[:].to_broadcast([128, NPIX]))

        a_col = small.tile([128, 1], F32, tag="a_col")
        nc.vector.tensor_reduce(out=a_col[:], in_=am_bc[:], axis=AX.X, op=OP.add)
        # a2 = [sum(am); 1] on partitions 0-1 (per-partition scalar for the
        # B = a2*brows + S assembly)
        a2 = small.tile([2, 1], F32, tag="a2")
        nc.vector.memset(a2[:, 0:1], 1.0)
        nc.scalar.activation(out=a2[0:1, 0:1], in_=a_col[0:1, 0:1], func=AF.Copy)

        S = small.tile([2, C], F32, tag="S")  # s-matvec PSUM rows, pre-bias
        B = small.tile([2, C], F32, tag="B")  # [s1; s2] rows (exact fp32)
        B_r = small.tile([3, C], F32R, tag="B_r")  # f32r copy + m row for PE
        scratch = dramp.tile([1, C], F32, tag="scratch")  # m transpose roundtrip

        # `passes` > 1 re-runs the whole pipeline for differential timing.
        for _pass in range(passes):
            with ExitStack() as pp:
                fme_pool = pp.enter_context(tc.tile_pool(name="fme", bufs=NCHUNK))
                vpool = pp.enter_context(tc.tile_pool(name="v", bufs=NCHUNK))
                big_pool = pp.enter_context(tc.tile_pool(name="big", bufs=2))
                u_pool = pp.enter_context(tc.tile_pool(name="u", bufs=NCHUNK))

                fme_tiles = []
                u_tiles = []
                v_tiles = []

                # ================= phase A ================================
                with ExitStack() as pa:
                    wv_pool = pa.enter_context(tc.tile_pool(name="wv", bufs=3))
                    wq_pool = pa.enter_context(tc.tile_pool(name="wq", bufs=4))
                    psv = pa.enter_context(
                        tc.tile_pool(name="psv", bufs=4, space="PSUM")
                    )
                    pss = pa.enter_context(
                        tc.tile_pool(name="pss", bufs=2, space="PSUM")
                    )

                    ps_s = [None] * 4
                    for r, (o0, ocnt) in enumerate(ROUNDS):
                        og = list(range(o0, o0 + ocnt))
                        # weight streams for this round
                        wvb = {}
                        for o in og:
                            wvb[o] = wv_pool.tile(
                                [128, NCHUNK, 128], F32R, tag="wv", name=f"wv{_pass}_{o}"
                            )
                            nc.sync.dma_start(out=wvb[o][:], in_=wvt[o].bitcast(F32R))
                        sq = SSCHED.get(r)
                        wq_tiles = {}
                        if sq is not None:
                            q, crange, _, _ = sq
                            for c in crange:
                                wq_tiles[c] = wq_pool.tile(
                                    [128, 512], F32R, tag="wq", name=f"wq{_pass}_{r}_{c}"
                                )
                                nc.scalar.dma_start(
                                    out=wq_tiles[c][:],
                                    in_=wqm[c][:, q * 512 : (q + 1) * 512].bitcast(F32R),
                                )
                            if sq[2]:  # start: allocate the quarter's psum
                                ps_s[q] = pss.tile(
                                    [2, 512], F32, tag="pss", name=f"pss{_pass}_{q}"
                                )
                        if r == 0:
                            # stream fm (resident fp32) + u reductions
                            for c in range(NCHUNK):
                                ft = fme_pool.tile([128, NPIX], F32R, tag="fme")
                                eng = nc.sync if (c % 2 == 0) else nc.gpsimd
                                eng.dma_start(
                                    out=ft[:],
                                    in_=fm[c * 128 : (c + 1) * 128, :].bitcast(F32R),
                                )
                                fme_tiles.append(ft)
                            for c in range(NCHUNK):
                                ut = u_pool.tile([128, 2], F32, tag="u")
                                utr = u_pool.tile([128, 2], F32R, tag="ur")
                                scr_a = big_pool.tile([128, NPIX], F32, tag="big")
                                nc.vector.tensor_mul(
                                    scr_a[:], fme_tiles[c][:].bitcast(F32), am_bc[:]
                                )
                                nc.vector.tensor_reduce(
                                    out=ut[:, 0:1], in_=scr_a[:], axis=AX.X, op=OP.add
                                )
                                nc.vector.tensor_reduce(
                                    out=ut[:, 1:2],
                                    in_=fme_tiles[c][:].bitcast(F32),
                                    axis=AX.X,
                                    op=OP.add,
                                )
                                nc.scalar.activation(
                                    out=utr[:], in_=ut[:], func=AF.Copy
                                )
                                u_tiles.append(utr)

                        pv = {}
                        for o in og:
                            pv[o] = [
                                psv.tile(
                                    [128, 512], F32, tag="pv",
                                    name=f"pv{_pass}_{o}_{h}",
                                )
                                for h in range(2)
                            ]
                        for c in range(NCHUNK):
                            for o in og:
                                for h in range(2):
                                    nc.tensor.matmul(
                                        pv[o][h][:],
                                        wvb[o][:, c, :],
                                        fme_tiles[c][:, h * 512 : (h + 1) * 512],
                                        start=(c == 0),
                                        stop=(c == NCHUNK - 1),
                                    )
                            if sq is not None and c in sq[1]:
                                q, crange, st, sp = sq
                                nc.tensor.matmul(
                                    ps_s[q][:],
                                    u_tiles[c][:],
                                    wq_tiles[c][:],
                                    start=(st and c == crange[0]),
                                    stop=(sp and c == crange[-1]),
                                )
                        # v extraction (bf16 + bias + ones cols)
                        for o in og:
                            vt = vpool.tile([128, NPIX + 2], mm_dt, tag="v")
                            nc.scalar.activation(
                                out=vt[:, 0:512], in_=pv[o][0][:], func=AF.Identity,
                                bias=bv_t[:, o : o + 1],
                            )
                            nc.scalar.activation(
                                out=vt[:, 512:1024], in_=pv[o][1][:], func=AF.Identity,
                                bias=bv_t[:, o : o + 1],
                            )
                            ones = vt[:, 1024:1026]
                            if mm_dt == F32R:
                                ones = ones.bitcast(F32)
                            nc.vector.memset(ones, 1.0)
                            v_tiles.append(vt)
                        if sq is not None and sq[3]:  # quarter done -> extract S
                            q = sq[0]
                            nc.scalar.activation(
                                out=S[0:2, q * 512 : (q + 1) * 512],
                                in_=ps_s[q][:],
                                func=AF.Copy,
                            )

                # ================= m phase ================================
                with ExitStack() as pm:
                    psm = pm.enter_context(
                        tc.tile_pool(name="psm", bufs=2, space="PSUM")
                    )
                    m_pool = pm.enter_context(tc.tile_pool(name="mcol", bufs=4))
                    # B[0:2] = a2 * brows + S   (s1, s2 rows with bias)
                    nc.vector.scalar_tensor_tensor(
                        out=B[:],
                        in0=brows_t[:],
                        scalar=a2[:, 0:1],
                        in1=S[:],
                        op0=OP.mult,
                        op1=OP.add,
                    )
                    nc.scalar.activation(
                        out=B_r[0:2, :], in_=B[:], func=AF.Copy
                    )
                    for i in range(NCHUNK):
                        pm_t = psm.tile([128, NH], F32, tag="psm", name=f"psm{_pass}_{i}")
                        nc.tensor.matmul(
                            pm_t[:],
                            B_r[0:2, i * 128 : (i + 1) * 128],
                            hull_t[:],
                            start=True,
                            stop=True,
                        )
                        mc = m_pool.tile([128, 1], F32, tag="mcol")
                        nc.vector.tensor_reduce(
                            out=mc[:], in_=pm_t[:], axis=AX.X, op=OP.max
                        )
                        nc.gpsimd.dma_start(
                            out=scratch[0:1, i * 128 : (i + 1) * 128], in_=mc[:]
                        )
                    nc.gpsimd.dma_start(
                        out=B_r[2:3, :], in_=scratch[:].bitcast(F32R)
                    )

                # ================= phase D ================================
                with ExitStack() as pd:
                    e_pool = pd.enter_context(tc.tile_pool(name="e", bufs=17))
                    z_pool = pd.enter_context(tc.tile_pool(name="z", bufs=4))
                    psc = pd.enter_context(
                        tc.tile_pool(name="psc", bufs=2, space="PSUM")
                    )
                    pso = pd.enter_context(
                        tc.tile_pool(name="pso", bufs=6, space="PSUM")
                    )

                    for ib in range(4):
                        isl = slice(ib * 512, (ib + 1) * 512)
                        eb = []
                        for j in range(NCHUNK):
                            sc = psc.tile(
                                [128, 512], F32, tag="psc", name=f"psc{_pass}_{ib}_{j}"
                            )
                            nc.tensor.matmul(
                                sc[:],
                                arows_t[:, j * 128 : (j + 1) * 128],
                                B_r[:, isl],
                                start=True,
                                stop=True,
                            )
                            et = e_pool.tile([128, 512], mm_dt, tag="e")
                            nc.scalar.activation(out=et[:], in_=sc[:], func=AF.Exp)
                            eb.append(et)
                        for ic in range(4):
                            ig = ib * 4 + ic
                            po = [
                                pso.tile(
                                    [128, b - a], F32, tag="po",
                                    name=f"po{_pass}_{ig}_{a}",
                                )
                                for (a, b) in NSPLIT
                            ]
                            # j-outer so the three n-chunk matmuls reuse the
                            # same stationary operand (one weight load per j)
                            for j in range(NCHUNK):
                                for nidx, (a, b) in enumerate(NSPLIT):
                                    nc.tensor.matmul(
                                        po[nidx][:],
                                        eb[j][:, ic * 128 : (ic + 1) * 128],
                                        v_tiles[j][:, a:b],
                                        start=(j == 0),
                                        stop=(j == NCHUNK - 1),
                                    )
                            rz = z_pool.tile([128, 1], F32, tag="rz")
                            nc.vector.reciprocal(rz[:], po[2][:, 340:341])
                            rzg = z_pool.tile([128, 1], F32, tag="rzg")
                            nc.vector.tensor_mul(rzg[:], rz[:], gam_bc[:])
                            ot = big_pool.tile([128, NPIX], F32, tag="big")
                            spans = [(0, 342, 0), (342, 684, 1), (684, 1024, 2)]
                            for a, b, nidx in spans:
                                nc.vector.scalar_tensor_tensor(
                                    out=ot[:, a:b],
                                    in0=po[nidx][:, 0 : b - a],
                                    scalar=rzg[:, 0:1],
                                    in1=fme_tiles[ig][:, a:b].bitcast(F32),
                                    op0=OP.mult,
                                    op1=OP.add,
                                )
                            nc.sync.dma_start(
                                out=out[ig * 128 : (ig + 1) * 128, :], in_=ot[:]
                            )

    nc.compile()
    return nc


def host_inputs(feature_map, attention_map, Wq, bq, Wk, bk, Wv, bv, gamma):
    """Shard + lay out inputs for the 8 cores; returns in_maps list."""
    f32 = np.float32
    B = feature_map.shape[0]
    fm = np.ascontiguousarray(feature_map.reshape(B, C, NPIX).astype(f32, copy=False))
    am = np.ascontiguousarray(
        attention_map.reshape(B, 1, NPIX).astype(f32, copy=False)
    )
    # blk[o, p, c, f] = Wv.T[c*128+p, o*128+f] = Wv[o*128+f, c*128+p]
    wvt_blk = np.ascontiguousarray(
        Wv.astype(f32, copy=False)
        .reshape(NCHUNK, 128, NCHUNK, 128)
        .transpose(0, 3, 2, 1)
    )
    # wqm[cb, p, o] = Wq[o, cb*128+p]
    wqm = np.ascontiguousarray(
        Wq.astype(f32, copy=False).T.reshape(NCHUNK, 128, C)
    )
    wk1 = Wk.reshape(C).astype(f32, copy=False)
    bk1 = bk.reshape(C).astype(f32, copy=False)
    bq1 = bq.reshape(C).astype(f32, copy=False)
    arows = np.ascontiguousarray(
        np.stack([wk1, bk1, -np.ones(C, f32)]).astype(f32)
    )
    brows = np.ascontiguousarray(
        np.stack([bq1, np.float32(NPIX) * bq1]).astype(f32)
    )
    bvcol = np.ascontiguousarray(
        bv.reshape(C).astype(f32, copy=False).reshape(NCHUNK, 128).T
    )

    # direction-sampled support points of {(Wk_j, bk_j)}: subset whose max
    # of (Wk_j * x + bk_j * y) is within r*(1-cos(pi/NH)) of the true max
    th = np.arange(NH, dtype=np.float64) * (2.0 * np.pi / NH)
    proj = np.cos(th)[:, None] * wk1[None, :] + np.sin(th)[:, None] * bk1[None, :]
    sel = np.argmax(proj, axis=1)
    hull = np.ascontiguousarray(np.stack([wk1[sel], bk1[sel]]).astype(f32))

    gam2 = np.ascontiguousarray(gamma.reshape(1, 1).astype(f32, copy=False))

    shared = dict(
        wvt=wvt_blk,
        wqm=wqm,
        arows=arows,
        brows=brows,
        bvcol=bvcol,
        hull=hull,
        gamma=gam2,
    )
    return [dict(fm=fm[b], am=am[b], **shared) for b in range(B)]


_NC_CACHE = {}


def get_nc(mm_dt=MM_DT):
    key = str(mm_dt)
    if key not in _NC_CACHE:
        _NC_CACHE[key] = build_nc(mm_dt)
    return _NC_CACHE[key]


def kernel(feature_map, attention_map, Wq, bq, Wk, bk, Wv, bv, gamma, **run_kwargs):
    from concourse.bass_utils import run_bass_kernel_spmd

    # plain numpy up front (jax-array inputs would run host prep on device)
    feature_map, attention_map, Wq, bq, Wk, bk, Wv, bv, gamma = (
        np.asarray(x) for x in (feature_map, attention_map, Wq, bq, Wk, bk, Wv, bv, gamma)
    )
    B, _, H, W = feature_map.shape
    in_maps = host_inputs(
        feature_map, attention_map, Wq, bq, Wk, bk, Wv, bv, gamma
    )
    nc = get_nc()
    res = run_bass_kernel_spmd(nc, in_maps, core_ids=list(range(NCORES)), **run_kwargs)
    out = np.stack([res.results[b]["out"].reshape(C, H, W) for b in range(B)])
    if run_kwargs:
        kernel.last_results = res
    return out.astype(np.float32, copy=False)


# revision 13
# speedup vs baseline: 1.2008x; 1.2008x over previous
"""Trainium2 Bass kernel for CrossAttentionModule (channel-wise attention).

Math restructuring
------------------
Reference (per sample b, with n = H*W pixels, C channels):
    q = Wq @ fm + bq            # [C, n]
    k = Wk * am + bk            # [C, n]  (rank-2 in the channel axis!)
    v = Wv @ fm + bv            # [C, n]
    scores[i, j] = <q[i, :], k[j, :]>
    out = softmax_j(scores) @ v
    result = gamma * out + fm

Because k[j, p] = Wk[j] * am[p] + bk[j]:
    scores[i, j] = s1[i] * Wk[j] + s2[i] * bk[j]
where
    s1 = Wq @ (fm @ am) + sum(am) * bq      # [C]
    s2 = Wq @ (fm @ 1)  + n * bq            # [C]

Device pipeline (per core = one sample, data-parallel over batch):
  phase A (8 rounds x 2 o-chunks, c-outer):
    round 0 streams fm (kept resident in SBUF, fp32); DVE computes the
    u = [fm@am, fm@1] reductions per c-chunk.  Every round runs the
    V = Wv@fm GEMM for its 2 o-chunks (f32r), accumulating over all 16
    c-chunks; v tiles written bf16 (+ones cols for Z).  The s matvec
    streams Wq^T as the f32r MOVING operand against the tiny stationary
    u (2 cols -> trivial weight loads), one 512-wide o-quarter per 2
    rounds, so the 16MB Wq read hides under the V GEMM.
  m phase: row max via direction-sampled support points of
    {(Wk_j, bk_j)} evaluated with tiny rank-2 PE matmuls; B = [s1; s2; m]
    rows assembled in SBUF (m via a small DRAM transpose roundtrip).
  phase D (4 i-blocks): scores^T tiles come from rank-3 PE matmuls
    A^T @ B with A = [wk; bk; -1] (no DVE work), ACT exp -> bf16 e
    tiles, then probs @ v accumulates over j on PE; epilogue divides by
    the ones-column Z, scales by gamma and adds the resident fm.
"""

import os
import sys

for _p in ("/opt/trn_rl_repo", "/root/.axon_site/_ro/trn_rl_repo"):
    if os.path.isdir(_p) and _p not in sys.path:
        sys.path.insert(0, _p)

from contextlib import ExitStack

import numpy as np

import concourse.bacc as bacc
import concourse.bass as bass
import concourse.mybir as mybir
import concourse.tile as tile

C = 2048
NPIX = 1024
NCORES = 8
NH = 64  # direction-sampled support points for the row max
NCHUNK = C // 128  # 16

F32 = mybir.dt.float32
F32R = mybir.dt.float32r
BF16 = mybir.dt.bfloat16
OP = mybir.AluOpType
AX = mybir.AxisListType
AF = mybir.ActivationFunctionType

# dtype of the probs/v operands for the P@V GEMM (bf16 halves SBUF and
# enables fast weight load; rel-err contribution ~6e-4).
MM_DT = BF16 if os.environ.get("CA_MM_DT", "bf16") == "bf16" else F32R

# n-chunk split of the 1026-wide (v | ones | pad) moving operand: each
# matmul output must fit one PSUM bank (<=512 fp32).  Column 1024 holds
# the ones-column (Z); 1025 is padding.
NSPLIT = [(0, 342), (342, 684), (684, 1026)]

# phase A rounds: (o_start, count) pairs; 2 o-chunks per round so V GEMM
# needs only 4 PSUM banks and can start after the first fm tile lands.
ROUNDS = [(2 * r, 2) for r in range(8)]
# s-matvec schedule: (round, quarter, c_range, start, stop)
SSCHED = {
    1: (0, range(0, 8), True, False),
    2: (0, range(8, 16), False, True),
    3: (1, range(0, 8), True, False),
    4: (1, range(8, 16), False, True),
    5: (2, range(0, 8), True, False),
    6: (2, range(8, 16), False, True),
    7: (3, range(0, 16), True, True),
}


def build_nc(mm_dt=MM_DT, passes=1):
    nc = bacc.Bacc("TRN2", target_bir_lowering=False)

    fm = nc.declare_dram_parameter("fm", [C, NPIX], F32, isOutput=False)
    am = nc.declare_dram_parameter("am", [1, NPIX], F32, isOutput=False)
    # weight blocks pre-swizzled on host: [o, p, c, f] = Wv.T[c*128+p, o*128+f]
    wvt = nc.declare_dram_parameter("wvt", [NCHUNK, 128, NCHUNK, 128], F32, isOutput=False)
    # wqm[cb, p, o] = Wq[o, cb*128+p]  (moving-operand layout, c on partitions)
    wqm = nc.declare_dram_parameter("wqm", [NCHUNK, 128, C], F32, isOutput=False)
    arows = nc.declare_dram_parameter("arows", [3, C], F32, isOutput=False)  # wk, bk, -1
    brows = nc.declare_dram_parameter("brows", [2, C], F32, isOutput=False)  # bq, n*bq
    bvcol = nc.declare_dram_parameter("bvcol", [128, NCHUNK], F32, isOutput=False)
    hull = nc.declare_dram_parameter("hull", [2, NH], F32, isOutput=False)
    gam = nc.declare_dram_parameter("gamma", [1, 1], F32, isOutput=False)
    out = nc.declare_dram_parameter("out", [C, NPIX], F32, isOutput=True)

    with ExitStack() as ctx:
        tc = ctx.enter_context(tile.TileContext(nc))
        small = ctx.enter_context(tc.tile_pool(name="small", bufs=1))
        dramp = ctx.enter_context(tc.tile_pool(name="dram", bufs=1, space="DRAM"))

        # ---- small persistent tiles -------------------------------------
        arows_t = small.tile([3, C], F32R, tag="arows")
        nc.gpsimd.dma_start(out=arows_t[:], in_=arows[:].bitcast(F32R))
        brows_t = small.tile([2, C], F32, tag="brows")
        nc.gpsimd.dma_start(out=brows_t[:], in_=brows[:])
        hull_t = small.tile([2, NH], F32R, tag="hull")
        nc.gpsimd.dma_start(out=hull_t[:], in_=hull[:].bitcast(F32R))
        bv_t = small.tile([128, NCHUNK], F32, tag="bv")
        nc.gpsimd.dma_start(out=bv_t[:], in_=bvcol[:])
        gam_bc = small.tile([128, 1], F32, tag="gam")
        nc.gpsimd.dma_start(out=gam_bc[:], in_=gam[:].to_broadcast([128, 1]))
        am_bc = small.tile([128, NPIX], F32, tag="am_bc")
        nc.gpsimd.dma_start(out=am_bc[:], in_=am[:].to_broadcast([128, NPIX]))

        a_col = small.tile([128, 1], F32, tag="a_col")
        nc.vector.tensor_reduce(out=a_col[:], in_=am_bc[:], axis=AX.X, op=OP.add)
        # a2 = [sum(am); 1] on partitions 0-1 (per-partition scalar for the
        # B = a2*brows + S assembly)
        a2 = small.tile([2, 1], F32, tag="a2")
        nc.vector.memset(a2[:, 0:1], 1.0)
        nc.scalar.activation(out=a2[0:1, 0:1], in_=a_col[0:1, 0:1], func=AF.Copy)

        B_r = small.tile([3, C], F32R, tag="B_r")  # [s1; s2; m] rows for PE
        scratch = dramp.tile([1, C], F32, tag="scratch")  # m transpose roundtrip

        fme_pool = ctx.enter_context(tc.tile_pool(name="fme", bufs=20))
        vpool = ctx.enter_context(tc.tile_pool(name="v", bufs=NCHUNK))
        big_pool = ctx.enter_context(tc.tile_pool(name="big", bufs=2))
        u_pool = ctx.enter_context(tc.tile_pool(name="u", bufs=2 * NCHUNK))

        # `passes` > 1 re-runs the whole pipeline for differential timing.
        for _pass in range(passes):
            with ExitStack() as pp:
                fme_tiles = []
                u_tiles = []
                v_tiles = []

                # ================= phase A ================================
                with ExitStack() as pa:
                    wv_pool = pa.enter_context(tc.tile_pool(name="wv", bufs=3))
                    wq_pool = pa.enter_context(tc.tile_pool(name="wq", bufs=4))
                    psv = pa.enter_context(
                        tc.tile_pool(name="psv", bufs=4, space="PSUM")
                    )
                    pss = pa.enter_context(
                        tc.tile_pool(name="pss", bufs=2, space="PSUM")
                    )

                    ps_s = [None] * 4
                    for r, (o0, ocnt) in enumerate(ROUNDS):
                        og = list(range(o0, o0 + ocnt))
                        # weight streams for this round
                        wvb = {}
                        for o in og:
                            wvb[o] = wv_pool.tile(
                                [128, NCHUNK, 128], F32R, tag="wv", name=f"wv{_pass}_{o}"
                            )
                            nc.sync.dma_start(out=wvb[o][:], in_=wvt[o].bitcast(F32R))
                        sq = SSCHED.get(r)
                        wq_tiles = {}
                        if sq is not None:
                            q, crange, _, _ = sq
                            for c in crange:
                                wq_tiles[c] = wq_pool.tile(
                                    [128, 512], F32R, tag="wq", name=f"wq{_pass}_{r}_{c}"
                                )
                                nc.scalar.dma_start(
                                    out=wq_tiles[c][:],
                                    in_=wqm[c][:, q * 512 : (q + 1) * 512].bitcast(F32R),
                                )
                            if sq[2]:  # start: allocate the quarter's psum
                                ps_s[q] = pss.tile(
                                    [2, 512], F32, tag="pss", name=f"pss{_pass}_{q}"
                                )
                        if r == 0:
                            # stream fm (resident fp32) + u reductions
                            for c in range(NCHUNK):
                                ft = fme_pool.tile([128, NPIX], F32R, tag="fme")
                                eng = nc.sync if (c % 2 == 0) else nc.gpsimd
                                eng.dma_start(
                                    out=ft[:],
                                    in_=fm[c * 128 : (c + 1) * 128, :].bitcast(F32R),
                                )
                                fme_tiles.append(ft)
                            for c in range(NCHUNK):
                                ut = u_pool.tile([128, 2], F32, tag="u")
                                utr = u_pool.tile([128, 2], F32R, tag="ur")
                                scr_a = big_pool.tile([128, NPIX], F32, tag="big")
                                nc.vector.tensor_mul(
                                    scr_a[:], fme_tiles[c][:].bitcast(F32), am_bc[:]
                                )
                                nc.vector.tensor_reduce(
                                    out=ut[:, 0:1], in_=scr_a[:], axis=AX.X, op=OP.add
                                )
                                nc.vector.tensor_reduce(
                                    out=ut[:, 1:2],
                                    in_=fme_tiles[c][:].bitcast(F32),
                                    axis=AX.X,
                                    op=OP.add,
                                )
                                nc.scalar.activation(
                                    out=utr[:], in_=ut[:], func=AF.Copy
                                )
                                u_tiles.append(utr)

                        pv = {}
                        for o in og:
                            pv[o] = [
                                psv.tile(
                                    [128, 512], F32, tag="pv",
                                    name=f"pv{_pass}_{o}_{h}",
                                )
                                for h in range(2)
                            ]
                        for c in range(NCHUNK):
                            for o in og:
                                for h in range(2):
                                    nc.tensor.matmul(
                                        pv[o][h][:],
                                        wvb[o][:, c, :],
                                        fme_tiles[c][:, h * 512 : (h + 1) * 512],
                                        start=(c == 0),
                                        stop=(c == NCHUNK - 1),
                                    )
                            if sq is not None and c in sq[1]:
                                q, crange, st, sp = sq
                                nc.tensor.matmul(
                                    ps_s[q][:],
                                    u_tiles[c][:],
                                    wq_tiles[c][:],
                                    start=(st and c == crange[0]),
                                    stop=(sp and c == crange[-1]),
                                )
                        # v extraction (bf16 + bias + ones cols)
                        for o in og:
                            vt = vpool.tile([128, NPIX + 2], mm_dt, tag="v")
                            nc.scalar.activation(
                                out=vt[:, 0:512], in_=pv[o][0][:], func=AF.Identity,
                                bias=bv_t[:, o : o + 1],
                            )
                            nc.scalar.activation(
                                out=vt[:, 512:1024], in_=pv[o][1][:], func=AF.Identity,
                                bias=bv_t[:, o : o + 1],
                            )
                            ones = vt[:, 1024:1026]
                            if mm_dt == F32R:
                                ones = ones.bitcast(F32)
                            nc.vector.memset(ones, 1.0)
                            v_tiles.append(vt)
                        if sq is not None and sq[3]:  # quarter done -> B_r rows
                            q = sq[0]
                            qs = slice(q * 512, (q + 1) * 512)
                            nc.vector.scalar_tensor_tensor(
                                out=B_r[0:2, qs],
                                in0=brows_t[:, qs],
                                scalar=a2[:, 0:1],
                                in1=ps_s[q][:],
                                op0=OP.mult,
                                op1=OP.add,
                            )

                # ================= m phase ================================
                with ExitStack() as pm:
                    psm = pm.enter_context(
                        tc.tile_pool(name="psm", bufs=1, space="PSUM")
                    )
                    m_pool = pm.enter_context(tc.tile_pool(name="mcol", bufs=1))
                    pm_t = psm.tile(
                        [128, NCHUNK, NH], F32, tag="psm", name=f"psm{_pass}"
                    )
                    for i in range(NCHUNK):
                        nc.tensor.matmul(
                            pm_t[:, i, :],
                            B_r[0:2, i * 128 : (i + 1) * 128],
                            hull_t[:],
                            start=(i % 8 == 0),
                            stop=(i % 8 == 7),
                            skip_group_check=True,
                        )
                    mc16 = m_pool.tile([128, NCHUNK], F32, tag="mcol")
                    nc.vector.tensor_reduce(
                        out=mc16[:], in_=pm_t[:], axis=AX.X, op=OP.max
                    )
                    for i in range(NCHUNK):
                        nc.gpsimd.dma_start(
                            out=scratch[0:1, i * 128 : (i + 1) * 128],
                            in_=mc16[:, i : i + 1],
                        )
                    nc.gpsimd.dma_start(
                        out=B_r[2:3, :], in_=scratch[:].bitcast(F32R)
                    )

                # ================= phase D ================================
                with ExitStack() as pd:
                    e_pool = pd.enter_context(tc.tile_pool(name="e", bufs=17))
                    z_pool = pd.enter_context(tc.tile_pool(name="z", bufs=4))
                    psc = pd.enter_context(
                        tc.tile_pool(name="psc", bufs=2, space="PSUM")
                    )
                    pso = pd.enter_context(
                        tc.tile_pool(name="pso", bufs=6, space="PSUM")
                    )

                    for ib in range(4):
                        isl = slice(ib * 512, (ib + 1) * 512)
                        eb = []
                        for j in range(NCHUNK):
                            sc = psc.tile(
                                [128, 512], F32, tag="psc", name=f"psc{_pass}_{ib}_{j}"
                            )
                            nc.tensor.matmul(
                                sc[:],
                                arows_t[:, j * 128 : (j + 1) * 128],
                                B_r[:, isl],
                                start=True,
                                stop=True,
                            )
                            et = e_pool.tile([128, 512], mm_dt, tag="e")
                            nc.scalar.activation(out=et[:], in_=sc[:], func=AF.Exp)
                            eb.append(et)
                        for ic in range(4):
                            ig = ib * 4 + ic
                            po = [
                                pso.tile(
                                    [128, b - a], F32, tag="po",
                                    name=f"po{_pass}_{ig}_{a}",
                                )
                                for (a, b) in NSPLIT
                            ]
                            # j-outer so the three n-chunk matmuls reuse the
                            # same stationary operand (one weight load per j)
                            for j in range(NCHUNK):
                                for nidx, (a, b) in enumerate(NSPLIT):
                                    nc.tensor.matmul(
                                        po[nidx][:],
                                        eb[j][:, ic * 128 : (ic + 1) * 128],
                                        v_tiles[j][:, a:b],
                                        start=(j == 0),
                                        stop=(j == NCHUNK - 1),
                                    )
                            rz = z_pool.tile([128, 1], F32, tag="rz")
                            nc.vector.reciprocal(rz[:], po[2][:, 340:341])
                            rzg = z_pool.tile([128, 1], F32, tag="rzg")
                            nc.vector.tensor_mul(rzg[:], rz[:], gam_bc[:])
                            ot = big_pool.tile([128, NPIX], F32, tag="big")
                            spans = [(0, 342, 0), (342, 684, 1), (684, 1024, 2)]
                            for a, b, nidx in spans:
                                nc.vector.scalar_tensor_tensor(
                                    out=ot[:, a:b],
                                    in0=po[nidx][:, 0 : b - a],
                                    scalar=rzg[:, 0:1],
                                    in1=fme_tiles[ig][:, a:b].bitcast(F32),
                                    op0=OP.mult,
                                    op1=OP.add,
                                )
                            nc.sync.dma_start(
                                out=out[ig * 128 : (ig + 1) * 128, :], in_=ot[:]
                            )

    nc.compile()
    return nc


def host_inputs(feature_map, attention_map, Wq, bq, Wk, bk, Wv, bv, gamma):
    """Shard + lay out inputs for the 8 cores; returns in_maps list."""
    f32 = np.float32
    B = feature_map.shape[0]
    fm = np.ascontiguousarray(feature_map.reshape(B, C, NPIX).astype(f32, copy=False))
    am = np.ascontiguousarray(
        attention_map.reshape(B, 1, NPIX).astype(f32, copy=False)
    )
    # blk[o, p, c, f] = Wv.T[c*128+p, o*128+f] = Wv[o*128+f, c*128+p]
    wvt_blk = np.ascontiguousarray(
        Wv.astype(f32, copy=False)
        .reshape(NCHUNK, 128, NCHUNK, 128)
        .transpose(0, 3, 2, 1)
    )
    # wqm[cb, p, o] = Wq[o, cb*128+p]
    wqm = np.ascontiguousarray(
        Wq.astype(f32, copy=False).T.reshape(NCHUNK, 128, C)
    )
    wk1 = Wk.reshape(C).astype(f32, copy=False)
    bk1 = bk.reshape(C).astype(f32, copy=False)
    bq1 = bq.reshape(C).astype(f32, copy=False)
    arows = np.ascontiguousarray(
        np.stack([wk1, bk1, -np.ones(C, f32)]).astype(f32)
    )
    brows = np.ascontiguousarray(
        np.stack([bq1, np.float32(NPIX) * bq1]).astype(f32)
    )
    bvcol = np.ascontiguousarray(
        bv.reshape(C).astype(f32, copy=False).reshape(NCHUNK, 128).T
    )

    # direction-sampled support points of {(Wk_j, bk_j)}: subset whose max
    # of (Wk_j * x + bk_j * y) is within r*(1-cos(pi/NH)) of the true max
    th = np.arange(NH, dtype=np.float64) * (2.0 * np.pi / NH)
    proj = np.cos(th)[:, None] * wk1[None, :] + np.sin(th)[:, None] * bk1[None, :]
    sel = np.argmax(proj, axis=1)
    hull = np.ascontiguousarray(np.stack([wk1[sel], bk1[sel]]).astype(f32))

    gam2 = np.ascontiguousarray(gamma.reshape(1, 1).astype(f32, copy=False))

    shared = dict(
        wvt=wvt_blk,
        wqm=wqm,
        arows=arows,
        brows=brows,
        bvcol=bvcol,
        hull=hull,
        gamma=gam2,
    )
    return [dict(fm=fm[b], am=am[b], **shared) for b in range(B)]


_NC_CACHE = {}


def get_nc(mm_dt=MM_DT):
    key = str(mm_dt)
    if key not in _NC_CACHE:
        _NC_CACHE[key] = build_nc(mm_dt)
    return _NC_CACHE[key]


def kernel(feature_map, attention_map, Wq, bq, Wk, bk, Wv, bv, gamma, **run_kwargs):
    from concourse.bass_utils import run_bass_kernel_spmd

    # plain numpy up front (jax-array inputs would run host prep on device)
    feature_map, attention_map, Wq, bq, Wk, bk, Wv, bv, gamma = (
        np.asarray(x) for x in (feature_map, attention_map, Wq, bq, Wk, bk, Wv, bv, gamma)
    )
    B, _, H, W = feature_map.shape
    in_maps = host_inputs(
        feature_map, attention_map, Wq, bq, Wk, bk, Wv, bv, gamma
    )
    nc = get_nc()
    res = run_bass_kernel_spmd(nc, in_maps, core_ids=list(range(NCORES)), **run_kwargs)
    out = np.stack([res.results[b]["out"].reshape(C, H, W) for b in range(B)])
    if run_kwargs:
        kernel.last_results = res
    return out.astype(np.float32, copy=False)
